# revision 37
# baseline (speedup 1.0000x reference)
"""Bass/Trainium2 kernel for nn_AdaptiveSparseReservoir (self-contained).

out[b, c] = relu(sum_k x[b, rows[k]] * values[k] for cols[k]==c  + bias[c])
  x [1024, 4096] f32; values [262144] f32; rows/cols [262144] i32;
  bias [4096] f32  ->  out [1024, 4096] f32

Strategy
--------
Densify the sparse COO kernel on the host into W [4096, 4096] (1.6%
density with unstructured support is far too dense for gather/scatter on
TRN2 — a dense bf16 TensorEngine matmul moves ~16x fewer bytes), then run
the dense matmul column-sharded across the 8 NeuronCores with NO
collectives: core i computes outT_i = relu(W[:, 512i:512(i+1)].T @ x.T + b_i).

Schedule (staggered bank finishes; calibrated-model single-shot 62.9us vs
67.4us for the previous all-banks-finish-together schedule):
- Phase 1 (k-tiles 0..K1-1): k-major over all 8 PSUM banks — matches the
  DMA stream rate (x+W arrive k-major), keeps weights serving 2 MMs each.
- Phase 2 (k-tiles K1..31): nt-serial — each 128-col bank-pair finishes
  its full contraction ~4.5 us apart, so 6 of the 8 bias+relu epilogues
  and output DMAs hide completely under remaining matmuls; the last
  bank-pair runs its two m-halves serially so even its first epilogue
  hides. Only the final m-half's epilogue (~split ACT||DVE, out-DMA on
  both HWDGE rings) remains on the tail (~3 us incl drain, vs ~7.5 us
  when all 8 banks finish inside the last k-tile). K1=22 is chosen so
  phase 2 only consumes x k-tiles the DMA stream has already delivered.
- The output is computed TRANSPOSED so the per-column bias lands on the
  PSUM partition axis: bias+relu is a single fused op per PSUM bank,
  alternating ScalarE `activation` / VectorE `tensor_scalar`; out is bf16.
- Inputs are host-packed partition-major; x streams as one 256KB chunk
  per k-tile (2KB/partition lines), W as 1-k then 4-k chunks, alternating
  across BOTH HWDGE rings — in the calibrated timeline model this keeps
  the PE stream stall-free after the head. x k0 is split per m-half and
  k0's MMs run mh-major so the first MM issues after ~256KB of DMA. The
  bias load rides behind the first data chunks instead of in front.
- A chain of cheap N=128 warm-up matmuls (garbage, discarded by k0's
  start=True) keeps the PE busy through the DMA head so the HAM clock
  gate un-throttles before the real stream begins.
- Epilogue computes are emitted before their dma_starts wherever both
  share an engine queue (a dma_start costs ~1.2us of queue issue time).
- TileContext's exit barrier is replaced by a drain-only tail: the Bass
  preamble sem_clears at the start of every execution, so the butterfly
  barrier + semaphore clears (~4 us) are dead weight.
"""

import os
import types

import numpy as np
import ml_dtypes

D_IN = 4096
UNITS = 4096
NNZ = 262144
BATCH = 1024
N_CORES = 8
N_SHARD = UNITS // N_CORES  # 512 output columns per core
K_TILES = D_IN // 128  # 32
N_TILES = N_SHARD // 128  # 4
M_HALVES = BATCH // 512  # 2
K1 = 22  # k-tiles in phase 1 (k-major); 32-K1 per-bank in phase 2
N_WARMUP = 14

_CACHE = {}


def _drain_only(self, tick_clock, wait_clock):
    """Tail = DMA/compute drain only; skip the butterfly barrier + sem
    clears (the Bass preamble sem_clears at the start of each execution,
    and NEFF completion already requires every engine queue to finish)."""
    from concourse.tile import ScopedClock

    drain_inst = self.nc.sync.drain()
    wait_clock.add_sem_waits(
        drain_inst.ins, ScopedClock({None: tick_clock.global_clock})
    )
    popped = self.nc._tile_sem_poison_stack.pop()
    assert popped is self._sem_poison


def _build(reps=1):
    """reps>1 emits the whole body `reps` times into one NEFF (shared SBUF
    tiles serialize the reps) — used by hwtime2.py for slope timing."""
    import concourse.mybir as mybir
    import concourse.tile as tile
    from concourse import bacc

    nc = bacc.Bacc("TRN2", target_bir_lowering=False, debug=False, num_devices=N_CORES)
    bf16 = mybir.dt.bfloat16
    f32 = mybir.dt.float32

    xT_ext = nc.declare_dram_parameter("xT", [128, K_TILES * 1024], bf16, isOutput=False)
    w_ext = nc.declare_dram_parameter("w", [128, K_TILES * 512], bf16, isOutput=False)
    b_ext = nc.declare_dram_parameter("bias", [128, N_TILES], f32, isOutput=False)
    out_ext = nc.declare_dram_parameter("out", [N_SHARD, BATCH], bf16, isOutput=True)

    tc_outer = tile.TileContext(nc)
    try:
        # verify the internals _drain_only touches exist in this concourse
        from concourse.tile import ScopedClock  # noqa: F401

        assert hasattr(tc_outer, "_drain_and_barrier")
        assert hasattr(nc, "_tile_sem_poison_stack")
        tc_outer._drain_and_barrier = types.MethodType(_drain_only, tc_outer)
    except Exception:
        pass  # stock barrier exit: ~4us slower, still correct
    with tc_outer as tc:
        with (
            tc.tile_pool(name="consts", bufs=1) as cpool,
            tc.tile_pool(name="xk", bufs=1) as xpool,
            tc.tile_pool(name="wk", bufs=1) as wpool,
            tc.tile_pool(name="osb", bufs=10) as opool,
            tc.tile_pool(name="psum", bufs=1, space="PSUM") as ppool,
        ):
            psum = [
                ppool.tile([128, 512], f32, tag=f"ps{i}", name=f"ps{i}")
                for i in range(N_TILES * M_HALVES)
            ]

            warm = cpool.tile([128, 128], bf16)
            # memset on the otherwise-idle Pool engine so the PE warm-up
            # chain isn't gated on DVE
            nc.gpsimd.memset(warm[:, :], 0)
            tbl_warm = cpool.tile([128, 1], f32)
            bias_sb = cpool.tile([128, N_TILES], f32)
            xts = xpool.tile([128, K_TILES * 1024], bf16, name="xts")
            wts = wpool.tile([128, K_TILES * 512], bf16, name="wts")

            def emit_warmups_and_dma():
                # PE warm-up chain against the HAM cold clock, sized to span
                # the DMA head; k=0's start=True clear discards the garbage.
                # N=128 keeps each link cheap so real MMs queue at most
                # ~107ns behind.
                for _ in range(N_WARMUP):
                    nc.tensor.matmul(
                        psum[0][:, 0:128], warm[:, :], warm[:, :],
                        start=True, stop=True,
                    )

                # interleave x/w chunks in k order, alternating HWDGE rings;
                # fine-grained first chunks (early PE start) — x k0 split
                # per m-half so the first MMs start after ~256KB in flight
                chunks = [("x0", 0, 0), ("w", 0, 1), ("x1", 0, 0)]
                # x: one k-tile (256KB, 2KB/partition) per chunk — keeps the
                # stream granular enough that the PE never outruns delivery;
                # w: 1-k chunks early, 4-k (2KB/partition) once ahead
                xbounds = list(range(1, K_TILES + 1))
                wbounds = [1, 2, 3, 4, 6] + list(range(8, K_TILES + 1, 4))
                xi = wi = 0
                while xi < len(xbounds) - 1 or wi < len(wbounds) - 1:
                    kx = xbounds[xi] if xi < len(xbounds) - 1 else K_TILES
                    kw = wbounds[wi] if wi < len(wbounds) - 1 else K_TILES
                    if kw <= kx and wi < len(wbounds) - 1:
                        chunks.append(("w", wbounds[wi], wbounds[wi + 1]))
                        wi += 1
                    else:
                        chunks.append(("x", xbounds[xi], xbounds[xi + 1]))
                        xi += 1
                for i, (kind, klo, khi) in enumerate(chunks):
                    eng = nc.sync if i % 2 == 0 else nc.scalar
                    if kind == "x":
                        eng.dma_start(
                            xts[:, klo * 1024 : khi * 1024],
                            xT_ext[:, klo * 1024 : khi * 1024],
                        )
                    elif kind == "x0":
                        eng.dma_start(xts[:, 0:512], xT_ext[:, 0:512])
                    elif kind == "x1":
                        eng.dma_start(xts[:, 512:1024], xT_ext[:, 512:1024])
                    else:
                        eng.dma_start(
                            wts[:, klo * 512 : khi * 512],
                            w_ext[:, klo * 512 : khi * 512],
                        )
                    if i == 3:
                        # bias rides behind the first data chunks — needed
                        # only at the epilogues, must not delay x/w bytes
                        nc.scalar.dma_start(bias_sb[:, :], b_ext[:, :])

                # trigger the Relu act-table load now (ACT is idle during
                # the stream); bacc hoists LoadActFuncSet before this
                # instruction, keeping ~1.3us off the epilogue critical path
                nc.scalar.activation(
                    tbl_warm[:, :], warm[:, 0:1], mybir.ActivationFunctionType.Relu
                )

            def mm(k, nt, mh, stop=False):
                nc.tensor.matmul(
                    psum[nt * M_HALVES + mh][:, :],
                    wts[:, k * 512 + nt * 128 : k * 512 + (nt + 1) * 128],
                    xts[:, k * 1024 + mh * 512 : k * 1024 + (mh + 1) * 512],
                    start=(k == 0),
                    stop=stop,
                )

            def epi_compute(nt, mh, lo, hi, on_act):
                """bias+relu for psum bank (nt,mh) cols [lo,hi) -> bf16 out
                tile; returns the tile for the caller to DMA out."""
                i = nt * M_HALVES + mh
                src = psum[i][:, lo:hi]
                ot = opool.tile(
                    [128, hi - lo], bf16, name=f"ot{i}_{lo}", tag=f"ot{i}_{lo}"
                )
                if on_act:
                    nc.scalar.activation(
                        ot[:, :],
                        src,
                        mybir.ActivationFunctionType.Relu,
                        bias=bias_sb[:, nt : nt + 1],
                    )
                else:
                    nc.vector.tensor_scalar(
                        ot[:, :],
                        src,
                        bias_sb[:, nt : nt + 1],
                        0.0,
                        mybir.AluOpType.add,
                        mybir.AluOpType.max,
                    )
                return ot

            def epi_dma(ot, nt, mh, lo, hi, on_sync):
                eng = nc.sync if on_sync else nc.scalar
                eng.dma_start(
                    out_ext[nt * 128 : (nt + 1) * 128, mh * 512 + lo : mh * 512 + hi],
                    ot[:, :],
                )

            def epilogue(nt, mh, lo, hi, on_act):
                ot = epi_compute(nt, mh, lo, hi, on_act)
                epi_dma(ot, nt, mh, lo, hi, on_sync=on_act)

            def emit_matmuls_and_epilogues():
                # k0 mh-major: the first 4 MMs need only x[k0, mh0] + w[k0]
                for mh in range(M_HALVES):
                    for nt in range(N_TILES):
                        mm(0, nt, mh)
                # phase 1: k-major, weights serve both m-halves
                for k in range(1, K1):
                    for nt in range(N_TILES):
                        for mh in range(M_HALVES):
                            mm(k, nt, mh)

                # phase 2: nt-serial — bank-pair nt finishes at ~(K1 +
                # (nt+1)*(32-K1))/32 of the stream; its fused bias+relu
                # epilogue and output DMA hide under bank-pairs nt+1..
                for nt in range(N_TILES - 1):
                    for k in range(K1, K_TILES):
                        for mh in range(M_HALVES):
                            mm(k, nt, mh, stop=(k == K_TILES - 1))
                    # hidden under bank-pairs nt+1..: full-tile epilogues
                    epilogue(nt, 0, 0, 512, on_act=True)
                    epilogue(nt, 1, 0, 512, on_act=False)
                # last bank-pair: run the two m-halves serially so mh0's
                # epilogue+DMA hide under mh1's ~2us k-run; only mh1's
                # epilogue (ACT||DVE, out-DMA on both rings) is on the tail
                nt = N_TILES - 1
                for k in range(K1, K_TILES):
                    mm(k, nt, 0, stop=(k == K_TILES - 1))
                epilogue(nt, 0, 0, 512, on_act=True)
                for k in range(K1, K_TILES):
                    mm(k, nt, 1, stop=(k == K_TILES - 1))
                # final epilogue: one full-width ACT op (splitting it across
                # ACT and DVE is a trap — the scheduler coalesces the
                # second piece's PE wait onto the first piece's engine
                # clock, serializing them in the emitted NEFF), then the
                # output leaves as two half-DMAs, one per HWDGE ring, which
                # run in parallel on silicon
                ot = epi_compute(nt, 1, 0, 512, on_act=True)
                epi_dma(ot[:, 0:256], nt, 1, 0, 256, on_sync=True)
                epi_dma(ot[:, 256:512], nt, 1, 256, 512, on_sync=False)

            for _ in range(reps):
                emit_warmups_and_dma()
                emit_matmuls_and_epilogues()

    nc.compile()
    return nc


def _get_nc():
    if "nc" not in _CACHE:
        _CACHE["nc"] = _build()
    return _CACHE["nc"]


def kernel(x, values, bias, rows, cols):
    from concourse.bass_utils import run_bass_kernel_spmd

    x = np.asarray(x, np.float32)
    values = np.asarray(values, np.float32)
    bias = np.asarray(bias, np.float32)
    rows = np.asarray(rows)
    cols = np.asarray(cols)

    # densify via bincount (vectorized scatter-add; duplicates accumulate)
    flat = rows.astype(np.int64) * UNITS + cols.astype(np.int64)
    W = np.bincount(flat, weights=values.astype(np.float64), minlength=D_IN * UNITS)
    W = W.reshape(D_IN, UNITS).astype(np.float32)

    # partition-major xT: xT_pm[p, k*1024 + m] = x[m, k*128 + p]
    xT16 = np.ascontiguousarray(x.T).astype(ml_dtypes.bfloat16)  # [D_IN, BATCH]
    xT_pm = np.ascontiguousarray(
        xT16.reshape(K_TILES, 128, BATCH).transpose(1, 0, 2).reshape(128, K_TILES * BATCH)
    )
    W16 = W.astype(ml_dtypes.bfloat16)

    in_maps = []
    for i in range(N_CORES):
        w_shard = W16[:, i * N_SHARD : (i + 1) * N_SHARD]  # [D_IN, 512]
        # partition-major W: w_pm[p, k*512 + n] = W[k*128 + p, n0 + n]
        w_pm = np.ascontiguousarray(
            w_shard.reshape(K_TILES, 128, N_SHARD)
            .transpose(1, 0, 2)
            .reshape(128, K_TILES * N_SHARD)
        )
        b_shard = np.ascontiguousarray(
            bias[i * N_SHARD : (i + 1) * N_SHARD].reshape(N_TILES, 128).T
        )
        in_maps.append({"xT": xT_pm, "w": w_pm, "bias": b_shard})

    nc = _get_nc()
    res = run_bass_kernel_spmd(nc, in_maps, list(range(N_CORES)))
    out = np.empty((BATCH, UNITS), np.float32)
    for i in range(N_CORES):
        out[:, i * N_SHARD : (i + 1) * N_SHARD] = (
            res.results[i]["out"].astype(np.float32).T
        )
    return out
